# revision 15
# baseline (speedup 1.0000x reference)
"""MidGCN forward on 8 Trainium2 NeuronCores (Bass/Tile, SPMD row-sharding).

Math (alpha = 0.5):
  DAD   = d_row * adj * d_col          (d = rsqrt of row/col sums)
  adj_f = (0.5*I - DAD)(I + DAD) = 0.5*I - 0.5*DAD - DAD@DAD
  h     = relu(adj_f @ (x @ W1))
  out   = log_softmax(adj_f @ (h @ W2) + b2)

Strategy vs the bf16 predecessor (269.5us):
  * The adj slab is resident in SBUF as fp8e4 (e4m3): halves the HBM
    load (8MB) and enables DoubleRow matmuls (two 128-deep k-tiles per
    instruction at 0.5 cycles/row): the two NxN passes drop ~4x.
  * d_col is folded into the narrow activations (z's), never into the
    slab: z_q = fp8(S * d_col * z) per pass, so the slab stays raw and
    single-quantized.  Scales S1/S2/Sv/Su keep each z at sigma ~0.7 in
    e4m3 range; the inverse scales fold into the per-row epilogue
    scalars.  Verified numerically: rel err ~5.7e-3 (tolerance 2e-2).
  * Colsum partials ride the slab DMA on DVE/ACT; rowsums via a
    DoubleRow fp8 ones-vector PE pass.  ReduceScatter (not AllReduce)
    delivers exactly the own-row d_col slice - no per-core addressing.
  * rsqrt = Exp(-0.5*Ln(x)): every ACT func used ({Copy,Relu,Exp,Ln})
    lives in one act table -> a single LoadActFuncSet for the kernel.
  * Each pass computes mt-halves 0-3 / 4-7 separately with per-half
    AllGathers; the consumer pass processes the k-parity of the first
    half first, hiding collective+DMA latency under live matmuls.
  * Epilogues use fused scalar_tensor_tensor (one DVE op per AXPY) and
    ACT Copy-with-scale for the fp8 quantizes straight out of PSUM.
"""

import numpy as np
import ml_dtypes

NCORE = 8
N = 8192
NF = 512
NH = 256
NC = 2
RPC = N // NCORE          # rows per core = 1024
KT = N // 128             # 64 contraction tiles
NQ = KT // 2              # 32 DoubleRow k-pairs
MT = RPC // 128           # 8 output row tiles per core
FT = NF // 128            # 4 k-tiles for x @ W1
HR = RPC // 2             # 512 rows per mt-half

S1 = 64.0                 # zq = fp8(S1 * d_col * zs)
S2 = 4096.0               # zt = fp8(psum1 * d_row*d_col*S2/S1)
SV = 16.0                 # zv = fp8(SV * d_col * y2)
SU = 512.0                # zu = fp8(psum_u * d_row*d_col*SU/SV)

_CACHE = {}


def _build(lite=False, sim=False):
    import concourse.bass as bass
    import concourse.mybir as mybir
    import concourse.tile as tile
    from concourse import bacc, masks
    from concourse.bass import ts

    BF = mybir.dt.bfloat16
    F32 = mybir.dt.float32
    FP8 = mybir.dt.float8e4
    AX = mybir.AxisListType
    OP = mybir.AluOpType
    AF = mybir.ActivationFunctionType
    PM = mybir.MatmulPerfMode

    nc = bacc.Bacc("TRN2", target_bir_lowering=False, debug=False,
                   num_devices=NCORE)

    adjT = nc.dram_tensor("adjT", [N, RPC], FP8, kind="ExternalInput")
    xT = nc.dram_tensor("xT", [NF, RPC], BF, kind="ExternalInput")
    w1 = nc.dram_tensor("w1", [NF, NH], BF, kind="ExternalInput")
    w2h = nc.dram_tensor("w2h", [NH, NC], BF, kind="ExternalInput")
    b2 = nc.dram_tensor("b2", [1, NC], F32, kind="ExternalInput")
    out = nc.dram_tensor("out", [RPC, NC], F32, kind="ExternalOutput")

    cs_in = nc.dram_tensor("cs_in", [N], F32)
    cs_rs = nc.dram_tensor("cs_rs", [RPC], F32)
    rs_dram = nc.dram_tensor("rs_dram", [RPC], F32)
    zq_in = nc.dram_tensor("zq_in", [RPC, NH], FP8)
    zq_out = nc.dram_tensor("zq_out", [N, NH], FP8, addr_space="Shared")
    zt_in = nc.dram_tensor("zt_in", [RPC, NH], FP8)
    zt_o = [nc.dram_tensor(f"zt_o{h}", [N // 2, NH], FP8,
                           addr_space="Shared") for h in range(2)]
    zv_in = nc.dram_tensor("zv_in", [RPC, NC], FP8)
    zv_o = [nc.dram_tensor(f"zv_o{h}", [N // 2, NC], FP8,
                           addr_space="Shared") for h in range(2)]
    zu_in = nc.dram_tensor("zu_in", [RPC, NC], FP8)
    zu_o = [nc.dram_tensor(f"zu_o{h}", [N // 2, NC], FP8,
                           addr_space="Shared") for h in range(2)]
    RG = [list(range(NCORE))]

    if lite:
        # I/O-identical null kernel: measures tunnel/dispatch overhead.
        with tile.TileContext(nc) as tc:
            with tc.tile_pool(name="p0", bufs=1) as p0:
                o = p0.tile([128, MT, NC], F32, tag="o")
                nc.vector.memset(o, 0.0)
                nc.sync.dma_start(
                    out=out[:].rearrange("(mt p) c -> p mt c", p=128), in_=o)
        nc.compile()
        return nc

    # k-pair order for passes consuming half-gathered z: the parity-0
    # pairs (kt%8 in 0..3) arrive with AllGather #1, parity-1 with #2.
    QORD = [q for q in range(NQ) if q % 4 < 2] + \
           [q for q in range(NQ) if q % 4 >= 2]

    def hslot(q):
        # (half, flat slot) of k-pair q in the per-half gather layout
        h = 0 if q % 4 < 2 else 1
        return h, 4 * (q // 4) + 2 * (q % 4) - 4 * h

    with tile.TileContext(nc) as tc:
        from contextlib import ExitStack
        with ExitStack() as ctx:
            p_one = ctx.enter_context(tc.tile_pool(name="p_one", bufs=1))
            p_rot = ctx.enter_context(tc.tile_pool(name="p_rot", bufs=2))

            # ---------- persistent SBUF ----------
            slab = p_one.tile([128, KT, RPC], FP8, tag="slab")
            zbq = p_one.tile([128, KT, NH], FP8, tag="zbq")
            zbt = [p_one.tile([128, KT // 2, NH], FP8, tag=f"zbt{h}",
                              name=f"zbt{h}") for h in range(2)]
            zvf = [p_one.tile([128, KT // 2, NC], FP8, tag=f"zvf{h}",
                              name=f"zvf{h}") for h in range(2)]
            zuf = [p_one.tile([128, KT // 2, NC], FP8, tag=f"zuf{h}",
                              name=f"zuf{h}") for h in range(2)]
            xT_sb = p_one.tile([128, FT, RPC], BF, tag="xT")
            w1_sb = p_one.tile([128, FT, NH], BF, tag="w1")
            w2_sb = p_one.tile([128, 2, NC], BF, tag="w2")
            b2_sb = p_one.tile([128, NC], F32, tag="b2")
            ident = p_one.tile([128, 128], BF, tag="ident")
            ones8 = p_one.tile([128, 2, 32], FP8, tag="ones")
            csp = p_one.tile([128, KT], F32, tag="csp")
            s_sb = p_one.tile([128, MT, NH], F32, tag="s")
            zq_sb = p_one.tile([128, MT, NH], FP8, tag="zq")
            zt_sb = p_one.tile([128, MT, NH], FP8, tag="zt")
            zv_sb = p_one.tile([128, MT, NC], FP8, tag="zv")
            zu_sb = p_one.tile([128, MT, NC], FP8, tag="zu")
            y_sb = p_one.tile([128, MT, NC], F32, tag="y")
            row_sb = p_one.tile([1, RPC], F32, tag="rowsb")
            rloc = p_one.tile([128, MT], F32, tag="rloc")
            drow = p_one.tile([128, MT], F32, tag="drow")
            dcl = p_one.tile([128, MT], F32, tag="dcl")
            ddt = p_one.tile([128, MT], F32, tag="ddt")
            q1 = p_one.tile([128, MT], F32, tag="q1")
            dd2 = p_one.tile([128, MT], F32, tag="dd2")
            dvq = p_one.tile([128, MT], F32, tag="dvq")
            du2 = p_one.tile([128, MT], F32, tag="du2")
            e1n = p_one.tile([128, MT], F32, tag="e1n")
            e2 = p_one.tile([128, MT], F32, tag="e2")
            eu = p_one.tile([128, MT], F32, tag="eu")
            ew = p_one.tile([128, MT], F32, tag="ew")
            mx_sb = p_one.tile([128, MT], F32, tag="mx")
            out_sb = p_one.tile([128, MT, NC], F32, tag="osb")

            masks.make_identity(nc, ident)
            nc.vector.memset(ones8, 1.0)

            # weights / x first on the DMA queue, then the slab
            nc.sync.dma_start(out=xT_sb, in_=xT[:].rearrange(
                "(kt p) m -> p kt m", p=128))
            nc.sync.dma_start(out=w1_sb, in_=w1[:].rearrange(
                "(kt p) n -> p kt n", p=128))
            nc.sync.dma_start(out=w2_sb, in_=w2h[:].rearrange(
                "(kh p) c -> p kh c", p=128))
            nc.sync.dma_start(out=b2_sb, in_=b2[:].to_broadcast([128, NC]))

            with ExitStack() as c1:
                pm = c1.enter_context(
                    tc.tile_pool(name="pm", bufs=4, space="PSUM"))

                # ---- s = x @ W1 first in PE program order; the psum->
                # sbuf copies are emitted inside the slab loop so the DVE
                # queue head starts on colsum immediately ----
                ps = {}
                for g in range(2):
                    mts = range(4 * g, 4 * g + 4)
                    for mt in mts:
                        ps[mt] = pm.tile([128, NH], F32, tag="pm",
                                         name=f"ps{mt}")
                    for kt in range(FT):
                        for mt in mts:
                            nc.tensor.matmul(ps[mt], xT_sb[:, kt, ts(mt, 128)],
                                             w1_sb[:, kt, :],
                                             start=kt == 0, stop=kt == FT - 1)

                # ---- slab load; colsum partials on DVE/ACT; rowsum on PE --
                with ExitStack() as c0:
                    pr_p = c0.enter_context(
                        tc.tile_pool(name="prp", bufs=4, space="PSUM"))
                    prow = [pr_p.tile([32, 256], F32, tag="pr",
                            name=f"pr{j}") for j in range(4)]
                    for c in range(8):
                        nc.sync.dma_start(
                            out=slab[:, 8 * c:8 * c + 8, :],
                            in_=adjT[ts(c, RPC), :].rearrange(
                                "(kt p) m -> p kt m", p=128))
                        nd = 5 if c == 7 else 4
                        k0 = 8 * c
                        nc.vector.tensor_reduce(
                            out=csp[:, k0:k0 + nd],
                            in_=slab[:, k0:k0 + nd, :],
                            axis=AX.X, op=OP.add)
                        for kt in range(k0 + nd, k0 + 8):
                            scr = p_rot.tile([128, RPC], FP8, tag="scr",
                                             name=f"scr{kt}")
                            nc.scalar.activation(
                                out=scr, in_=slab[:, kt, :], func=AF.Copy,
                                accum_out=csp[:, kt:kt + 1])
                        if c == 1:
                            for mt in range(MT):
                                nc.vector.tensor_copy(s_sb[:, mt, :], ps[mt])
                        for q in range(4 * c, 4 * c + 4):
                            for j in range(4):
                                nc.tensor.matmul(
                                    prow[j], ones8,
                                    slab[:, 2 * q:2 * q + 2, ts(j, 256)],
                                    start=q == 0, stop=q == NQ - 1,
                                    perf_mode=PM.DoubleRow)
                    for j in range(4):
                        nc.vector.tensor_copy(row_sb[0:1, ts(j, 256)],
                                              prow[j][0:1, :])

                # d_row/d_col = exp(-0.5 ln(sum)): group Ln's then Exp's
                # so the act table switches only twice.
                nc.sync.dma_start(out=rs_dram[:], in_=row_sb[0:1, :])
                nc.sync.dma_start(
                    out=rloc,
                    in_=rs_dram[:].rearrange("(mt p) -> p mt", p=128))

                # ---- colsum store via PE transpose (contiguous (kt p)) ----
                with ExitStack() as ct:
                    ptc = ct.enter_context(
                        tc.tile_pool(name="ptc", bufs=1, space="PSUM"))
                    identf = p_one.tile([128, 128], F32, tag="identf")
                    masks.make_identity(nc, identf)
                    ctp = ptc.tile([KT, 128], F32, tag="ctp")
                    nc.tensor.transpose(ctp, csp, identf)
                    cst = p_one.tile([KT, 128], F32, tag="cst")
                    nc.vector.tensor_copy(cst, ctp)
                nc.sync.dma_start(
                    out=cs_in[:].rearrange("(kt p) -> kt p", kt=KT), in_=cst)
                if sim:
                    nc.sync.dma_start(out=cs_rs[:], in_=cs_in[0:RPC])
                else:
                    nc.gpsimd.collective_compute(
                        "ReduceScatter", OP.add, replica_groups=RG,
                        ins=[cs_in[:]], outs=[cs_rs[:]])
                nc.sync.dma_start(
                    out=dcl, in_=cs_rs[:].rearrange("(mt p) -> p mt", p=128))
                nc.scalar.activation(out=drow, in_=rloc, func=AF.Ln)
                nc.scalar.activation(out=dcl, in_=dcl, func=AF.Ln)
                nc.scalar.activation(out=drow, in_=drow, func=AF.Exp,
                                     scale=-0.5)
                nc.scalar.activation(out=dcl, in_=dcl, func=AF.Exp,
                                     scale=-0.5)

                # per-row scalar vectors
                nc.vector.tensor_tensor(ddt, drow, dcl, op=OP.mult)
                nc.vector.tensor_scalar_mul(q1, dcl, S1)
                nc.vector.tensor_scalar_mul(dd2, ddt, S2 / S1)
                nc.vector.tensor_scalar_mul(dvq, dcl, SV)
                nc.vector.tensor_scalar_mul(du2, ddt, SU / SV)
                nc.vector.tensor_scalar_mul(e1n, drow, -1.0 / S1)
                nc.vector.tensor_scalar_mul(e2, drow, -2.0 / S2)
                nc.vector.tensor_scalar_mul(eu, drow, -0.5 / SV)
                nc.vector.tensor_scalar_mul(ew, drow, -1.0 / SU)

                # zq = fp8(S1 * d_col * zs); gather; load k-major
                for mt in range(MT):
                    nc.vector.tensor_scalar(zq_sb[:, mt, :], s_sb[:, mt, :],
                                            q1[:, mt:mt + 1], None,
                                            op0=OP.mult)
                nc.sync.dma_start(
                    out=zq_in[:].rearrange("(mt p) n -> p mt n", p=128),
                    in_=zq_sb)
                if sim:
                    nc.sync.dma_start(out=zq_out[0:RPC, :], in_=zq_in[:])
                else:
                    nc.gpsimd.collective_compute(
                        "AllGather", OP.bypass, replica_groups=RG,
                        ins=[zq_in[:]], outs=[zq_out[:]])
                for c in range(4):
                    nc.sync.dma_start(
                        out=zbq[:, 16 * c:16 * c + 16, :],
                        in_=zq_out[ts(c, 2048), :].rearrange(
                            "(kt p) n -> p kt n", p=128))

                # ---------- pass 1: psum1 = adj @ zq, by mt-halves ----------
                for hf in range(2):
                    mts = range(4 * hf, 4 * hf + 4)
                    pp = {mt: pm.tile([128, NH], F32, tag="pm",
                                      name=f"p1_{mt}") for mt in mts}
                    for q in range(NQ):
                        for mt in mts:
                            nc.tensor.matmul(
                                pp[mt], slab[:, 2 * q:2 * q + 2, ts(mt, 128)],
                                zbq[:, 2 * q:2 * q + 2, :],
                                start=q == 0, stop=q == NQ - 1,
                                perf_mode=PM.DoubleRow)
                    # zt = fp8(psum1 * dd2) -> store half -> gather half
                    for mt in mts:
                        nc.vector.tensor_scalar(zt_sb[:, mt, :], pp[mt],
                                                dd2[:, mt:mt + 1], None,
                                                op0=OP.mult)
                    nc.sync.dma_start(
                        out=zt_in[ts(hf, HR), :].rearrange(
                            "(mt p) n -> p mt n", p=128),
                        in_=zt_sb[:, 4 * hf:4 * hf + 4, :])
                    if sim:
                        nc.sync.dma_start(out=zt_o[hf][0:HR, :],
                                          in_=zt_in[ts(hf, HR), :])
                    else:
                        nc.gpsimd.collective_compute(
                            "AllGather", OP.bypass, replica_groups=RG,
                            ins=[zt_in[ts(hf, HR), :]], outs=[zt_o[hf][:]])
                    for cc in range(2):
                        nc.sync.dma_start(
                            out=zbt[hf][:, 16 * cc:16 * cc + 16, :],
                            in_=zt_o[hf][ts(cc, 2048), :].rearrange(
                                "(f p) n -> p f n", p=128))
                    # A = s - T = s + e1n * psum1 (in place in s_sb)
                    for mt in mts:
                        nc.vector.scalar_tensor_tensor(
                            s_sb[:, mt, :], pp[mt], e1n[:, mt:mt + 1],
                            s_sb[:, mt, :], op0=OP.mult, op1=OP.add)

                # ---------- pass 2 + layer-1 epilogue ----------
                with ExitStack() as c2:
                    ptr = c2.enter_context(
                        tc.tile_pool(name="ptr", bufs=2, space="PSUM"))
                    pv = c2.enter_context(
                        tc.tile_pool(name="pv", bufs=1, space="PSUM"))
                    for hf in range(2):
                        mts = range(4 * hf, 4 * hf + 4)
                        pp = {mt: pm.tile([128, NH], F32, tag="pm",
                                          name=f"p2_{mt}") for mt in mts}
                        for qi, q in enumerate(QORD):
                            for mt in mts:
                                h_, f_ = hslot(q)
                                nc.tensor.matmul(
                                    pp[mt],
                                    slab[:, 2 * q:2 * q + 2, ts(mt, 128)],
                                    zbt[h_][:, f_:f_ + 2, :],
                                    start=qi == 0, stop=qi == NQ - 1,
                                    perf_mode=PM.DoubleRow)
                        for mt in mts:
                            # h2 = relu(A + e2*psum2); y2 = h2 @ (W2/2)
                            h2p = p_rot.tile([128, NH], F32, tag="h2p",
                                             bufs=3)
                            nc.vector.scalar_tensor_tensor(
                                h2p, pp[mt], e2[:, mt:mt + 1], s_sb[:, mt, :],
                                op0=OP.mult, op1=OP.add)
                            hp = p_rot.tile([128, NH], BF, tag="hp", bufs=3)
                            nc.vector.tensor_scalar_max(hp, h2p, 0.0)
                            psv = pv.tile([128, NC], F32, tag="pv")
                            for kh in range(2):
                                pstr = ptr.tile([128, 128], BF, tag="ptr")
                                nc.tensor.transpose(pstr, hp[:, ts(kh, 128)],
                                                    ident)
                                hT = p_rot.tile([128, 128], BF, tag="hT",
                                                bufs=3)
                                nc.vector.tensor_copy(hT, pstr)
                                nc.tensor.matmul(psv, hT, w2_sb[:, kh, :],
                                                 start=kh == 0, stop=kh == 1)
                            nc.vector.tensor_scalar_mul(y_sb[:, mt, :],
                                                        psv, 0.5)
                            nc.vector.tensor_scalar(zv_sb[:, mt, :], psv,
                                                    dvq[:, mt:mt + 1], None,
                                                    op0=OP.mult)
                        nc.sync.dma_start(
                            out=zv_in[ts(hf, HR), :].rearrange(
                                "(mt p) c -> p mt c", p=128),
                            in_=zv_sb[:, 4 * hf:4 * hf + 4, :])
                        if sim:
                            nc.sync.dma_start(out=zv_o[hf][0:HR, :],
                                              in_=zv_in[ts(hf, HR), :])
                        else:
                            nc.gpsimd.collective_compute(
                                "AllGather", OP.bypass, replica_groups=RG,
                                ins=[zv_in[ts(hf, HR), :]],
                                outs=[zv_o[hf][:]])
                        nc.sync.dma_start(
                            out=zvf[hf],
                            in_=zv_o[hf][:].rearrange("(f p) c -> p f c",
                                                      p=128))

            # ---------- narrow passes ----------
            with ExitStack() as c3:
                pnar = c3.enter_context(
                    tc.tile_pool(name="pnar", bufs=8, space="PSUM"))

                # pass 3: psum_u = adj @ zv
                for hf in range(2):
                    mts = range(4 * hf, 4 * hf + 4)
                    pu = {mt: pnar.tile([128, NC], F32, tag="pu",
                                        name=f"pu{mt}") for mt in mts}
                    for qi, q in enumerate(QORD):
                        for mt in mts:
                            h_, f_ = hslot(q)
                            nc.tensor.matmul(
                                pu[mt], slab[:, 2 * q:2 * q + 2, ts(mt, 128)],
                                zvf[h_][:, f_:f_ + 2, :],
                                start=qi == 0, stop=qi == NQ - 1,
                                perf_mode=PM.DoubleRow)
                    for mt in mts:
                        nc.vector.tensor_scalar(zu_sb[:, mt, :], pu[mt],
                                                du2[:, mt:mt + 1], None,
                                                op0=OP.mult)
                    nc.sync.dma_start(
                        out=zu_in[ts(hf, HR), :].rearrange(
                            "(mt p) c -> p mt c", p=128),
                        in_=zu_sb[:, 4 * hf:4 * hf + 4, :])
                    if sim:
                        nc.sync.dma_start(out=zu_o[hf][0:HR, :],
                                          in_=zu_in[ts(hf, HR), :])
                    else:
                        nc.gpsimd.collective_compute(
                            "AllGather", OP.bypass, replica_groups=RG,
                            ins=[zu_in[ts(hf, HR), :]], outs=[zu_o[hf][:]])
                    nc.sync.dma_start(
                        out=zuf[hf],
                        in_=zu_o[hf][:].rearrange("(f p) c -> p f c",
                                                  p=128))
                    # y += eu * psum_u  (= 0.5*y2 - 0.5*DAD@y2 so far)
                    for mt in mts:
                        nc.vector.scalar_tensor_tensor(
                            y_sb[:, mt, :], pu[mt], eu[:, mt:mt + 1],
                            y_sb[:, mt, :], op0=OP.mult, op1=OP.add)

                # fold the bias in while the gather flies
                for mt in range(MT):
                    nc.vector.tensor_add(y_sb[:, mt, :], y_sb[:, mt, :],
                                         b2_sb)

                # pass 4: psum_w = adj @ zu ; G = y + ew*psum_w; log_softmax
                # (grouped epilogue: all DVE prep, then Exp block, Ln block)
                pw = {}
                for hf in range(2):
                    mts = range(4 * hf, 4 * hf + 4)
                    for mt in mts:
                        pw[mt] = pnar.tile([128, NC], F32, tag="pu",
                                           name=f"pw{mt}")
                    for qi, q in enumerate(QORD):
                        for mt in mts:
                            h_, f_ = hslot(q)
                            nc.tensor.matmul(
                                pw[mt], slab[:, 2 * q:2 * q + 2, ts(mt, 128)],
                                zuf[h_][:, f_:f_ + 2, :],
                                start=qi == 0, stop=qi == NQ - 1,
                                perf_mode=PM.DoubleRow)
                sm_all = p_one.tile([128, MT], F32, tag="sm_all")
                lg_all = p_one.tile([128, MT], F32, tag="lg_all")
                for mt in range(MT):
                    nc.vector.scalar_tensor_tensor(
                        out_sb[:, mt, :], pw[mt], ew[:, mt:mt + 1],
                        y_sb[:, mt, :], op0=OP.mult, op1=OP.add)
                nc.vector.tensor_reduce(out=mx_sb, in_=out_sb,
                                        axis=AX.X, op=OP.max)
                for mt in range(MT):
                    nc.vector.tensor_scalar(out_sb[:, mt, :],
                                            out_sb[:, mt, :],
                                            mx_sb[:, mt:mt + 1], None,
                                            op0=OP.subtract)
                for mt in range(MT):
                    ex_t = p_rot.tile([128, NC], F32, tag="ex")
                    nc.scalar.activation(out=ex_t, in_=out_sb[:, mt, :],
                                         func=AF.Exp,
                                         accum_out=sm_all[:, mt:mt + 1])
                nc.scalar.activation(out=lg_all, in_=sm_all, func=AF.Ln)
                for mt in range(MT):
                    nc.vector.tensor_scalar(out_sb[:, mt, :],
                                            out_sb[:, mt, :],
                                            lg_all[:, mt:mt + 1], None,
                                            op0=OP.subtract)
                nc.sync.dma_start(
                    out=out[:].rearrange("(mt p) c -> p mt c", p=128),
                    in_=out_sb)

    nc.compile()
    return nc


def _get_nc(lite=False):
    key = "nc_lite" if lite else "nc"
    if key not in _CACHE:
        _CACHE[key] = _build(lite=lite)
    return _CACHE[key]


def _prep_in_maps(x, adj, W1, W2, b2):
    bf = ml_dtypes.bfloat16
    e4 = ml_dtypes.float8_e4m3fn
    f32 = np.float32
    x = np.asarray(x, f32)
    adj = np.asarray(adj, f32)
    w1 = np.asarray(W1, f32).astype(bf)
    w2h = (0.5 * np.asarray(W2, f32)).astype(bf)
    b2v = np.asarray(b2, f32).reshape(1, NC)
    in_maps = []
    for i in range(NCORE):
        rows = slice(i * RPC, (i + 1) * RPC)
        in_maps.append({
            "adjT": adj[rows, :].T.astype(e4),   # one fused copy+cast
            "xT": x[rows, :].T.astype(bf),
            "w1": w1, "w2h": w2h, "b2": b2v,
        })
    return in_maps


def _run(x, adj, W1, W2, b2, trace=False, lite=False, in_maps=None):
    from concourse.bass_utils import run_bass_kernel_spmd
    nc = _get_nc(lite=lite)
    if in_maps is None:
        in_maps = _prep_in_maps(x, adj, W1, W2, b2)
    res = run_bass_kernel_spmd(nc, in_maps, core_ids=list(range(NCORE)),
                               trace=trace)
    out = np.concatenate([r["out"] for r in res.results], axis=0)
    return out, res


def kernel(x, adj, W1, W2, b2):
    out, _ = _run(x, adj, W1, W2, b2, trace=False)
    return out
